# revision 1
# baseline (speedup 1.0000x reference)
"""Canny edge detector on 8 Trainium2 NeuronCores.

Strategy (pure data/spatial parallel, per sharding hint):
 - Shard the 2048-row image over 8 cores (256 output rows each) with a
   5-row halo on each side (2 blur + 1 sobel + 1 NMS + 1 hysteresis).
 - Inside each core: columns-on-partitions layout.  Partition p owns
   output columns [16p, 16p+16) and stores a 26-wide window
   [16p-5, 16p+21) so that EVERY stencil (horizontal and vertical) is a
   pure free-dimension AP offset.  No cross-partition communication, no
   PE, no PSUM: only DVE / GpSimd / ACT streaming ops.
 - The host pre-pads columns (2048 -> 2058) and halo rows with zeros so
   conv zero-padding semantics come for free and all 8 cores run the
   same SPMD program.
 - Math pipeline (all fp32, faithful to the reference):
     bh  = 5-tap horizontal gaussian on img
     vb  = 5-tap vertical gaussian on bh
     t1  = vertical [1,2,1] of vb;  t2 = vertical [1,0,-1] of vb
     gx  = horizontal [1,0,-1] of t1;  gy = horizontal [1,2,1] of t2
     m_c = sqrt(gx^2+gy^2); g = sum_c m_c; sgx = sum_c gx; sgy = sum_c gy
     axis classification via |sgy| vs tan(22.5/67.5)*|sgx| and sign(sgx*sgy)
     nms+thresholds fused: cc = max of the 2 neighbors along the axis;
       hp = g > max(cc, high);  lm = g > max(cc, nextbefore(low))
     hysteresis: out = lm & max3x3(hp)   (binary planes in fp16)
"""

import numpy as np

_COMPILED = {}

H = 2048
W = 2048
HALO = 5
ROWS_PER_CORE = H // 8            # 256
SHARD_ROWS = ROWS_PER_CORE + 2 * HALO   # 266
PADW = W + 2 * HALO               # 2058
N_CHUNK = 128                     # output rows per chunk
CHUNKS = [(r, r + N_CHUNK) for r in range(0, ROWS_PER_CORE, N_CHUNK)]


def _build(low, high):
    import concourse.bass as bass
    import concourse.bacc as bacc
    import concourse.mybir as mybir
    from concourse.tile import TileContext

    f32 = mybir.dt.float32
    Alu = mybir.AluOpType
    Act = mybir.ActivationFunctionType

    g5 = np.exp(-0.5 * (np.arange(5) - 2.0) ** 2).astype(np.float32)
    ga = float(g5[0])
    gb = float(g5[1])
    t1c = float(np.float32(np.tan(np.deg2rad(np.float64(22.5)))))
    t2c = float(np.float32(np.tan(np.deg2rad(np.float64(67.5)))))

    nc = bacc.Bacc()
    x = nc.dram_tensor("x", [3, SHARD_ROWS, PADW], f32, kind="ExternalInput")
    out = nc.dram_tensor("out", [ROWS_PER_CORE, W], f32, kind="ExternalOutput")

    with TileContext(nc) as tc:
        with tc.tile_pool(name="io", bufs=2) as iop, tc.tile_pool(
            name="pl", bufs=1
        ) as pool:
            for (r0, r1) in CHUNKS:
                N = r1 - r0
                R = N + 10          # img/bh rows
                RV = N + 6          # vb rows
                RT = N + 4          # t/g rows
                RN = N + 2          # nms rows

                gpl = pool.tile([128, RT, 20], f32, tag="g")
                sgx = pool.tile([128, RT, 20], f32, tag="sgx")
                sgy = pool.tile([128, RT, 20], f32, tag="sgy")

                for c in range(3):
                    img = iop.tile([128, R, 26], f32, tag="img")
                    src = bass.AP(
                        x, c * SHARD_ROWS * PADW + r0 * PADW,
                        [[16, 128], [PADW, R], [1, 26]],
                    )
                    nc.sync.dma_start(out=img[:], in_=src)

                    s1 = pool.tile([128, R, 22], f32, tag="tA")
                    s2 = pool.tile([128, R, 22], f32, tag="tB")
                    bh1 = pool.tile([128, R, 22], f32, tag="tC")
                    bh = pool.tile([128, R, 22], f32, tag="tD")
                    # horizontal 5-tap gaussian [ga, gb, 1, gb, ga]
                    nc.vector.tensor_tensor(s1[:], img[:, :, 1:23], img[:, :, 3:25], Alu.add)
                    nc.vector.tensor_tensor(s2[:], img[:, :, 0:22], img[:, :, 4:26], Alu.add)
                    nc.vector.scalar_tensor_tensor(
                        bh1[:], s1[:], gb, img[:, :, 2:24], Alu.mult, Alu.add)
                    nc.vector.scalar_tensor_tensor(
                        bh[:], s2[:], ga, bh1[:], Alu.mult, Alu.add)

                    v1 = pool.tile([128, RV, 22], f32, tag="tA")
                    v2 = pool.tile([128, RV, 22], f32, tag="tB")
                    vb1 = pool.tile([128, RV, 22], f32, tag="tC")
                    vb = pool.tile([128, RV, 22], f32, tag="tE")
                    # vertical 5-tap gaussian
                    nc.vector.tensor_tensor(v1[:], bh[:, 1:RV + 1, :], bh[:, 3:RV + 3, :], Alu.add)
                    nc.vector.tensor_tensor(v2[:], bh[:, 0:RV, :], bh[:, 4:RV + 4, :], Alu.add)
                    nc.vector.scalar_tensor_tensor(
                        vb1[:], v1[:], gb, bh[:, 2:RV + 2, :], Alu.mult, Alu.add)
                    nc.vector.scalar_tensor_tensor(
                        vb[:], v2[:], ga, vb1[:], Alu.mult, Alu.add)

                    u = pool.tile([128, RT, 22], f32, tag="tA")
                    t1 = pool.tile([128, RT, 22], f32, tag="tB")
                    t2 = pool.tile([128, RT, 22], f32, tag="tC")
                    # vertical sobel components
                    nc.vector.tensor_tensor(u[:], vb[:, 0:RT, :], vb[:, 2:RT + 2, :], Alu.add)
                    nc.vector.scalar_tensor_tensor(
                        t1[:], vb[:, 1:RT + 1, :], 2.0, u[:], Alu.mult, Alu.add)
                    nc.vector.tensor_tensor(t2[:], vb[:, 0:RT, :], vb[:, 2:RT + 2, :], Alu.subtract)

                    gx = sgx if c == 0 else pool.tile([128, RT, 20], f32, tag="tD")
                    gy = sgy if c == 0 else pool.tile([128, RT, 20], f32, tag="tE")
                    w2 = pool.tile([128, RT, 20], f32, tag="tF")
                    # horizontal sobel components
                    nc.vector.tensor_tensor(gx[:], t1[:, :, 0:20], t1[:, :, 2:22], Alu.subtract)
                    nc.vector.tensor_tensor(w2[:], t2[:, :, 0:20], t2[:, :, 2:22], Alu.add)
                    nc.vector.scalar_tensor_tensor(
                        gy[:], t2[:, :, 1:21], 2.0, w2[:], Alu.mult, Alu.add)

                    q1 = pool.tile([128, RT, 20], f32, tag="tA")
                    q2 = pool.tile([128, RT, 20], f32, tag="tB")
                    r2 = pool.tile([128, RT, 20], f32, tag="tC")
                    m = gpl if c == 0 else pool.tile([128, RT, 20], f32, tag="tF")
                    nc.scalar.activation(q1[:], gx[:], Act.Square)
                    nc.scalar.activation(q2[:], gy[:], Act.Square)
                    nc.vector.tensor_tensor(r2[:], q1[:], q2[:], Alu.add)
                    nc.scalar.activation(m[:], r2[:], Act.Sqrt)

                    if c > 0:
                        nc.vector.tensor_tensor(gpl[:], gpl[:], m[:], Alu.add)
                        nc.vector.tensor_tensor(sgx[:], sgx[:], gx[:], Alu.add)
                        nc.vector.tensor_tensor(sgy[:], sgy[:], gy[:], Alu.add)

                # ---- NMS ----
                u8 = mybir.dt.uint8
                rr = pool.tile([128, RN, 18], f32, tag="cand")
                ss = pool.tile([128, RN, 18], f32, tag="cand2")
                m0 = pool.tile([128, RN, 18], u8, tag="mk0")
                m2 = pool.tile([128, RN, 18], u8, tag="mk1")
                d = pool.tile([128, RN, 18], f32, tag="tE")
                dpos = pool.tile([128, RN, 18], u8, tag="mk2")
                nc.scalar.activation(rr[:], sgy[:, 1:RN + 1, 1:19], Act.Abs)
                nc.scalar.activation(ss[:], sgx[:, 1:RN + 1, 1:19], Act.Abs)
                nc.vector.scalar_tensor_tensor(m0[:], ss[:], t1c, rr[:], Alu.mult, Alu.is_ge)
                nc.vector.scalar_tensor_tensor(m2[:], ss[:], t2c, rr[:], Alu.mult, Alu.is_le)
                nc.vector.tensor_tensor(
                    d[:], sgx[:, 1:RN + 1, 1:19], sgy[:, 1:RN + 1, 1:19], Alu.mult)
                nc.vector.tensor_scalar(dpos[:], d[:], 0.0, None, Alu.is_ge)

                cand = pool.tile([128, RN, 18], f32, tag="cand")
                cc = pool.tile([128, RN, 18], f32, tag="cc")
                # base: c3 = max(SW, NE); overwrite with c1/c2/c0 by priority
                nc.vector.tensor_tensor(
                    cand[:], gpl[:, 2:RN + 2, 2:20], gpl[:, 0:RN, 0:18], Alu.max)  # c1 SE/NW
                nc.vector.tensor_tensor(
                    cc[:], gpl[:, 2:RN + 2, 0:18], gpl[:, 0:RN, 2:20], Alu.max)    # c3 SW/NE
                nc.vector.copy_predicated(cc[:], dpos[:], cand[:])
                cand2 = pool.tile([128, RN, 18], f32, tag="cand2")
                nc.vector.tensor_tensor(
                    cand2[:], gpl[:, 2:RN + 2, 1:19], gpl[:, 0:RN, 1:19], Alu.max)  # c2 S/N
                nc.vector.copy_predicated(cc[:], m2[:], cand2[:])
                cand3 = pool.tile([128, RN, 18], f32, tag="cand")
                nc.vector.tensor_tensor(
                    cand3[:], gpl[:, 1:RN + 1, 2:20], gpl[:, 1:RN + 1, 0:18], Alu.max)  # c0 E/W
                nc.vector.copy_predicated(cc[:], m0[:], cand3[:])

                f16 = mybir.dt.float16
                hp = pool.tile([128, RN, 18], f16, tag="tF")
                lm = pool.tile([128, N, 16], f32, tag="cand")
                lowx = float(np.nextafter(np.float32(low), np.float32(0.0)))
                nc.vector.scalar_tensor_tensor(
                    hp[:], cc[:], high, gpl[:, 1:RN + 1, 1:19], Alu.max, Alu.is_lt)
                nc.vector.scalar_tensor_tensor(
                    lm[:], cc[:, 1:N + 1, 1:17], lowx, gpl[:, 2:RN, 2:18],
                    Alu.max, Alu.is_lt)

                rm1 = pool.tile([128, RN, 16], f16, tag="cc2")
                rm = pool.tile([128, RN, 16], f16, tag="cand2")
                cm1 = pool.tile([128, N, 16], f16, tag="cc2")
                cm = pool.tile([128, N, 16], f16, tag="nmsCM")
                nc.vector.tensor_tensor(rm1[:], hp[:, :, 0:16], hp[:, :, 2:18], Alu.max)
                nc.vector.tensor_tensor(rm[:], rm1[:], hp[:, :, 1:17], Alu.max)
                nc.vector.tensor_tensor(cm1[:], rm[:, 0:N, :], rm[:, 2:RN, :], Alu.max)
                nc.vector.tensor_tensor(cm[:], cm1[:], rm[:, 1:N + 1, :], Alu.max)

                outt = iop.tile([128, N, 16], f32, tag="out")
                nc.vector.tensor_tensor(outt[:], lm[:], cm[:], Alu.mult)
                dst = bass.AP(out, r0 * W, [[16, 128], [W, N], [1, 16]])
                nc.sync.dma_start(out=dst, in_=outt[:])

    nc.finalize()
    return nc


def _get_compiled(low, high):
    key = (low, high)
    if key not in _COMPILED:
        _COMPILED[key] = _build(low, high)
    return _COMPILED[key]


def kernel(img, threshold1, threshold2, _trace=False):
    from concourse import bass_utils

    t1 = float(np.asarray(threshold1))
    t2 = float(np.asarray(threshold2))
    low, high = min(t1, t2), max(t1, t2)

    x = np.ascontiguousarray(np.asarray(img, dtype=np.float32)[0])  # [3,H,W]
    # pad columns with HALO zeros on both sides
    xp = np.zeros((3, H + 2 * HALO, PADW), dtype=np.float32)
    xp[:, HALO:HALO + H, HALO:HALO + W] = x

    in_maps = []
    for k in range(8):
        shard = np.ascontiguousarray(xp[:, k * ROWS_PER_CORE:k * ROWS_PER_CORE + SHARD_ROWS, :])
        in_maps.append({"x": shard})

    nc = _get_compiled(low, high)
    res = bass_utils.run_bass_kernel_spmd(nc, in_maps, core_ids=list(range(8)),
                                          trace=_trace)

    full = np.zeros((1, 1, H, W), dtype=np.float32)
    for k in range(8):
        full[0, 0, k * ROWS_PER_CORE:(k + 1) * ROWS_PER_CORE, :] = res.results[k]["out"]
    # reference forces image borders to zero
    full[:, :, 0, :] = 0.0
    full[:, :, -1, :] = 0.0
    full[:, :, :, 0] = 0.0
    full[:, :, :, -1] = 0.0
    if _trace:
        kernel._last_results = res
    return full

